# revision 5
# baseline (speedup 1.0000x reference)
"""GCN classifier on 8 TRN2 NeuronCores — 4-bit adjacency, SBUF-resident,
decoded to fp8 subnormal codes.

Math (reference):
    h1  = relu(adj @ (X @ W1) + b1)        [N, D]
    h2  = relu(adj @ (h1 @ W2) + b2)       [N, D]
    h3  = relu(h2 @ Wm1 + bm1)             [N, D]
    out = h3 @ Wm2 + bm2                   [N, 1]

Key ideas vs the uint8 predecessor:
  * adj is quantized to 4 bits (q = rint(a*15/amax), rel err ~2e-3 at the
    output) and packed 4 nibbles per u16.  The whole per-core shard is
    16.8 MB — it fits in SBUF, so it is DMA'd from HBM ONCE and both GCN
    layers unpack from the resident copy.  HBM adj traffic drops 4x vs
    uint8-streamed-twice.
  * fp8e4 (e4m3) bit patterns 0x00..0x0F decode linearly to n * 2^-9
    (subnormals + first binade), and the PE multiplies them exactly
    (hardware-verified).  So unpacking a nibble to an fp8 operand is a
    single AND — no magic-number offset, no colsum correction:
        ev pass:  (v & 0x0F0F)        -> two fp8 bytes per u16 lane
        od pass:  (v >> 4) & 0x0F0F   -> two more
    The 2^9 * amax/15 scale folds into the PSUM-evacuation activation.
  * Matmuls keep the predecessor's column-tiled structure (lhsT = Y
    k-blocks at PE tile positions (0,0)/(0,64); ev/od node halves on PSUM
    partition halves) which measures ~268 ns per k-block (all four
    512-wide matmuls overlap in the array).

Sharding: 1D row partition of adj (2048 rows/core), shard pre-transposed
on host to [N, P] nibbles.  All big matmuls contract over the SBUF
partition axis:

    layer 1:  Z1.T = (XW1).T @ B_c    (lhsT = host-computed XW1 k-blocks)
    gather:   per-core g = (h1 @ W2) blocks (PE-transposed), AllGather fp16
    layer 2:  Z2.T = G.T @ B_c
    head:     h3.T = relu(Wm1.T @ h2.T + bm1); out.T = Wm2.T @ h3.T + bm2

Layout bookkeeping: nibble-packing along P puts local nodes (4j..4j+3) in
lane j; the ev pass emits nodes (4j, 4j+2) at byte positions (2j, 2j+1)
and the od pass (4j+1, 4j+3) — i.e. outputs come out column-permuted
[evens | odds], identical to the predecessor.  The contraction (k) rows
of B_c and XW1 are host-permuted per 128-block to [evens | odds] so
layer-2's gathered G blocks line up.  The host unpermutes the final
[1, P] outputs.
"""

import numpy as np

N = 16384
D = 64
N_CORES = 8
P = N // N_CORES          # 2048 local nodes per core
KB = N // 128             # 128 contraction blocks
T_PER_CORE = P // 128     # 16 local row-blocks per core
LW = P // 4               # 512 u16 lanes per packed row
SLAB_KB = 8               # k-blocks per resident-load DMA slab
CH = 4                    # k-blocks unpacked per DVE chunk
EO_BUFS = 3               # unpacked ev/od chunk tiles in flight

_cache = {}


def _build(reps=1, use_collective=True, ablate=None):
    import concourse.bass as bass  # noqa: F401
    import concourse.mybir as mybir
    import concourse.tile as tile
    from concourse import bacc

    f32 = mybir.dt.float32
    fp16 = mybir.dt.float16
    fp8 = mybir.dt.float8e4
    u16 = mybir.dt.uint16

    AND = mybir.AluOpType.bitwise_and
    SHR = mybir.AluOpType.logical_shift_right
    relu = mybir.ActivationFunctionType.Relu
    ident = mybir.ActivationFunctionType.Identity
    copy = mybir.ActivationFunctionType.Copy

    nc = bacc.Bacc("TRN2", target_bir_lowering=False, debug=False,
                   num_devices=N_CORES)

    adjq = nc.dram_tensor("adjq", [N, LW], u16, kind="ExternalInput")
    yb = nc.dram_tensor("yb", [128, KB, D], fp16, kind="ExternalInput")
    w2 = nc.dram_tensor("w2", [D, D], fp16, kind="ExternalInput")
    wm1 = nc.dram_tensor("wm1", [D, D], fp16, kind="ExternalInput")
    wm2 = nc.dram_tensor("wm2", [D, 1], fp16, kind="ExternalInput")
    b1 = nc.dram_tensor("b1", [128, 1], f32, kind="ExternalInput")
    b2 = nc.dram_tensor("b2", [128, 1], f32, kind="ExternalInput")
    bm1 = nc.dram_tensor("bm1", [128, 1], f32, kind="ExternalInput")
    bm2 = nc.dram_tensor("bm2", [128, 1], f32, kind="ExternalInput")
    sc = nc.dram_tensor("sc", [128, 1], f32, kind="ExternalInput")  # 512*amax/15
    out = nc.dram_tensor("out", [2, P // 2], f32, kind="ExternalOutput")

    hb_ins = [nc.dram_tensor(f"hb_in{r}", [128, T_PER_CORE * D], fp16)
              for r in range(reps)]
    hb_outs = [nc.dram_tensor(f"hb_out{r}", [N_CORES * 128, T_PER_CORE * D],
                              fp16, addr_space="Shared")
               for r in range(reps)]

    n_slabs = KB // SLAB_KB
    n_chunks = KB // CH
    HP = P // 2                          # 1024
    HL = LW // 2                         # 256 u16 lanes per node-half

    with tile.TileContext(nc) as tc:
        with (
            tc.tile_pool(name="rq", bufs=1) as rq,       # resident packed adj
            tc.tile_pool(name="eo", bufs=EO_BUFS) as eo,
            tc.tile_pool(name="wq", bufs=1) as wq,       # yt / g_all (shared)
            tc.tile_pool(name="wpool", bufs=1) as wpool,
            tc.tile_pool(name="hv", bufs=1) as hv,
            tc.tile_pool(name="gpool", bufs=2) as gpool,
            tc.tile_pool(name="opool", bufs=1) as opool,
            tc.tile_pool(name="pacc", bufs=2, space="PSUM") as pacc,
            tc.tile_pool(name="psmall", bufs=1, space="PSUM") as psmall,
        ):
            # ---- constants (both partition halves) ----
            w2t = wpool.tile([128, D], fp16, tag="w2")
            nc.gpsimd.dma_start(w2t[0:D, :], w2[:])
            nc.gpsimd.dma_start(w2t[D:128, :], w2[:])
            wm1t = wpool.tile([128, D], fp16, tag="wm1")
            nc.gpsimd.dma_start(wm1t[0:D, :], wm1[:])
            nc.gpsimd.dma_start(wm1t[D:128, :], wm1[:])
            wm2t = wpool.tile([128, 1], fp16, tag="wm2")
            nc.gpsimd.dma_start(wm2t[0:D, :], wm2[:])
            nc.gpsimd.dma_start(wm2t[D:128, :], wm2[:])
            b1t = wpool.tile([128, 1], f32, tag="b1")
            nc.gpsimd.dma_start(b1t[:], b1[:])
            b2t = wpool.tile([128, 1], f32, tag="b2")
            nc.gpsimd.dma_start(b2t[:], b2[:])
            bm1t = wpool.tile([128, 1], f32, tag="bm1")
            nc.gpsimd.dma_start(bm1t[:], bm1[:])
            bm2t = wpool.tile([128, 1], f32, tag="bm2")
            nc.gpsimd.dma_start(bm2t[:], bm2[:])
            sct = wpool.tile([128, 1], f32, tag="sc")
            nc.gpsimd.dma_start(sct[:], sc[:])

            # ---- resident packed adjacency: loaded once, used twice ----
            rt = rq.tile([128, KB, LW], u16, tag="rq")
            if ablate == "dma":
                nc.vector.memset(rt[:, 0, :], 0x1234)
            else:
                for s in range(n_slabs):
                    src = adjq[s * SLAB_KB * 128:(s + 1) * SLAB_KB * 128, :]
                    nc.sync.dma_start(
                        rt[:, s * SLAB_KB:(s + 1) * SLAB_KB, :],
                        src.rearrange("(p n) i -> p n i", p=128))

            def big_layer(lhsT_of, psum_t, et0=None):
                """Unpack resident nibbles chunk-by-chunk; accumulate into
                psum_t [128, HP]: partitions 0:64 take the even-node output
                columns (ev), 64:128 the odd-node columns (od)."""
                for c in range(n_chunks):
                    if et0 is not None:
                        et = et0
                    else:
                        et = eo.tile([128, CH, 2 * LW], u16, tag="eo")
                        nc.vector.tensor_scalar(
                            et[:, :, 0:LW], rt[:, c * CH:(c + 1) * CH, :],
                            0x0F0F, None, AND)
                        if ablate != "dve1":
                            nc.vector.tensor_scalar(
                                et[:, :, LW:2 * LW],
                                rt[:, c * CH:(c + 1) * CH, :],
                                4, 0x0F0F, SHR, AND)
                    for j in range(CH):
                        kb = c * CH + j
                        first = kb == 0
                        last = kb == KB - 1
                        if ablate == "mm" and not (first or last):
                            continue
                        for ic in range(2):
                            nc.tensor.matmul(
                                psum_t[0:D, ic * 512:(ic + 1) * 512],
                                lhsT_of(kb),
                                et[:, j, ic * HL:(ic + 1) * HL].bitcast(fp8),
                                start=first, stop=last, tile_position=(0, 0))
                        for ic in range(2):
                            nc.tensor.matmul(
                                psum_t[D:128, ic * 512:(ic + 1) * 512],
                                lhsT_of(kb),
                                et[:, j, LW + ic * HL:LW + (ic + 1) * HL]
                                .bitcast(fp8),
                                start=first, stop=last, tile_position=(0, 64))

            def evac_relu(psum_t, bias_t, htag):
                """h block = relu(sc * psum + bias), kept in the two-half
                block layout [128, HP]."""
                ht = hv.tile([128, HP], fp16, tag=htag)
                nc.scalar.activation(ht[:], psum_t[:], relu, bias=bias_t[:],
                                     scale=sct[:])
                return ht

            for _rep in range(reps):
                hb_in = hb_ins[_rep]
                hb_out = hb_outs[_rep]

                yt = wq.tile([128, KB, D], fp16, tag="wq")
                nc.scalar.dma_start(yt[:], yb[:])

                et0 = None
                if ablate == "static":
                    et0 = gpool.tile([128, CH, 2 * LW], u16, tag="et0")
                    nc.vector.tensor_scalar(
                        et0[:, :, 0:LW], rt[:, 0:CH, :], 0x0F0F, None, AND)
                    nc.vector.tensor_scalar(
                        et0[:, :, LW:2 * LW], rt[:, 0:CH, :],
                        4, 0x0F0F, SHR, AND)

                # ---- layer 1 ----
                pz = pacc.tile([128, HP], f32, tag="acc")
                big_layer(lambda kb: yt[:, kb, :], pz, et0)
                h1 = evac_relu(pz, b1t, "h1")

                # ---- local g blocks: g = h1 @ W2, PE-transposed ----
                gl = gpool.tile([128, T_PER_CORE, D], fp16, tag="gl")
                for t in range(T_PER_CORE):
                    pg = psmall.tile([128, D], f32, tag="pg")
                    nc.tensor.matmul(pg[0:D, :],
                                     h1[0:D, t * D:(t + 1) * D], w2t[0:D, :],
                                     start=True, stop=True,
                                     tile_position=(0, 0))
                    nc.tensor.matmul(pg[D:128, :],
                                     h1[D:128, t * D:(t + 1) * D],
                                     w2t[D:128, :], start=True, stop=True,
                                     tile_position=(64, 64))
                    nc.scalar.activation(gl[:, t, :], pg[:], copy)
                nc.scalar.dma_start(hb_in[:], gl[:])

                # ---- AllGather g ----
                if use_collective:
                    nc.gpsimd.collective_compute(
                        "AllGather",
                        mybir.AluOpType.bypass,
                        replica_groups=[list(range(N_CORES))],
                        ins=[hb_in.ap().opt()],
                        outs=[hb_out.ap().opt()],
                    )
                g_all = wq.tile([128, KB, D], fp16, tag="wq")
                for src in range(N_CORES):
                    if use_collective:
                        nc.scalar.dma_start(
                            g_all[:, src * T_PER_CORE:(src + 1) * T_PER_CORE,
                                  :],
                            hb_out[src * 128:(src + 1) * 128, :].rearrange(
                                "p (t d) -> p t d", d=D))
                    else:
                        nc.scalar.dma_start(
                            g_all[:, src * T_PER_CORE:(src + 1) * T_PER_CORE,
                                  :],
                            hb_in[:].rearrange("p (t d) -> p t d", d=D))

                # ---- layer 2 ----
                pz2 = pacc.tile([128, HP], f32, tag="acc")
                big_layer(lambda kb: g_all[:, kb, :], pz2, et0)
                h2 = evac_relu(pz2, b2t, "h2")

                # ---- head (block layout: halves on partition halves) ----
                p3 = pacc.tile([128, HP], f32, tag="acc")
                for ic in range(2):
                    nc.tensor.matmul(p3[0:D, ic * 512:(ic + 1) * 512],
                                     wm1t[0:D, :],
                                     h2[0:D, ic * 512:(ic + 1) * 512],
                                     start=True, stop=True,
                                     tile_position=(0, 0))
                    nc.tensor.matmul(p3[D:128, ic * 512:(ic + 1) * 512],
                                     wm1t[D:128, :],
                                     h2[D:128, ic * 512:(ic + 1) * 512],
                                     start=True, stop=True,
                                     tile_position=(64, 64))
                h3 = hv.tile([128, HP], fp16, tag="h3")
                nc.scalar.activation(h3[:], p3[:], relu, bias=bm1t[:])

                outsb = opool.tile([128, HP], f32, tag="out")
                for ic in range(2):
                    po = psmall.tile([128, 512], f32, tag="po")
                    nc.tensor.matmul(po[0:1, :], wm2t[0:D, :],
                                     h3[0:D, ic * 512:(ic + 1) * 512],
                                     start=True, stop=True,
                                     tile_position=(0, 0))
                    nc.tensor.matmul(po[D:D + 1, :], wm2t[D:128, :],
                                     h3[D:128, ic * 512:(ic + 1) * 512],
                                     start=True, stop=True,
                                     tile_position=(64, 64))
                    nc.scalar.activation(outsb[0:1, ic * 512:(ic + 1) * 512],
                                         po[0:1, :], ident, bias=bm2t[0:1, :])
                    nc.scalar.activation(
                        outsb[D:D + 1, ic * 512:(ic + 1) * 512],
                        po[D:D + 1, :], ident, bias=bm2t[D:D + 1, :])
                nc.scalar.dma_start(out[:], outsb[0:128:D, :])

    nc.compile()
    return nc


def _perm_k():
    """Per-128-block [evens | odds] permutation of row indices."""
    i = np.arange(N)
    b, r = i // 128, i % 128
    return b * 128 + np.where(r < 64, 2 * r, 2 * (r - 64) + 1)


_SIGMA = None


def _sigma():
    """Output column m -> local node index."""
    global _SIGMA
    if _SIGMA is None:
        m = np.arange(P)
        _SIGMA = np.where(m < P // 2, 2 * m, 2 * (m - P // 2) + 1)
    return _SIGMA


def _shard_adjq(adj, scale_inv, pk, c):
    """Quantize to 4 bits + transpose + k-permute + slab-swizzle one
    core's shard.

    Column packing: u16 lane j holds local nodes 4j..4j+3 at nibble
    positions 0/4/8/12, so the device's ev pass (v & 0x0F0F) emits nodes
    (4j, 4j+2) and the od pass ((v>>4) & 0x0F0F) nodes (4j+1, 4j+3) —
    the [evens | odds] column order the rest of the pipeline assumes.

    Row swizzle: storage row s*G + p*SLAB_KB + j holds logical (permuted)
    row s*G + j*128 + p, so each partition's slab read is one contiguous
    SLAB_KB*LW*2-byte run (G = SLAB_KB*128 rows per slab)."""
    block = adj[c * P:(c + 1) * P, :]                  # [P, N] f32
    q16 = np.empty((N, LW), dtype=np.uint16)
    step = 2048
    for k0 in range(0, N, step):
        sub = block[:, k0:k0 + step].T * scale_inv     # [step, P]
        q = np.rint(sub).astype(np.uint16)             # 4-bit values
        q16[k0:k0 + step, :] = (q[:, 0::4] | (q[:, 1::4] << 4)
                                | (q[:, 2::4] << 8) | (q[:, 3::4] << 12))
    r = np.arange(N)
    G = SLAB_KB * 128
    s_, rem = r // G, r % G
    p_, j_ = rem // SLAB_KB, rem % SLAB_KB
    src = pk[s_ * G + j_ * 128 + p_]
    return np.ascontiguousarray(q16[src, :])           # [N, LW]


def _prep_inputs(adj, features, W1, b1, W2, b2, Wm1, bm1, Wm2, bm2):
    from concurrent.futures import ThreadPoolExecutor

    adj = np.asarray(adj, dtype=np.float32)
    amax = float(adj.max())
    scale_inv = 15.0 / amax
    pk = _perm_k()

    y = np.asarray(features, np.float32) @ np.asarray(W1, np.float32)
    y = y[pk, :]                                       # [N, 64] permuted
    ybv = np.ascontiguousarray(
        y.reshape(KB, 128, D).transpose(1, 0, 2)).astype(np.float16)

    def dup(v):
        v = np.asarray(v, np.float32).reshape(-1)
        if v.size == 1:
            return np.full((128, 1), v[0], np.float32)
        return np.concatenate([v, v]).reshape(128, 1)

    common = {
        "yb": ybv,
        "w2": np.asarray(W2, np.float16),
        "wm1": np.asarray(Wm1, np.float16),
        "wm2": np.asarray(Wm2, np.float16).reshape(D, 1),
        "b1": dup(b1),
        "b2": dup(b2),
        "bm1": dup(bm1),
        "bm2": dup(bm2),
        "sc": np.full((128, 1), 512.0 * amax / 15.0, np.float32),
    }
    with ThreadPoolExecutor(max_workers=8) as ex:
        shards = list(ex.map(
            lambda c: _shard_adjq(adj, scale_inv, pk, c), range(N_CORES)))
    return [dict(common, adjq=shards[c]) for c in range(N_CORES)]


def _run(in_maps, **kw):
    from concourse.bass_utils import run_bass_kernel_spmd

    if "nc" not in _cache:
        _cache["nc"] = _build()
    res = run_bass_kernel_spmd(_cache["nc"], in_maps,
                               core_ids=list(range(N_CORES)), **kw)
    sig = _sigma()
    full = np.empty((N,), np.float32)
    for c in range(N_CORES):
        full[c * P + sig] = res.results[c]["out"].reshape(-1)
    return full[:, None], res


def kernel(adj, features, W1, b1, W2, b2, Wm1, bm1, Wm2, bm2):
    in_maps = _prep_inputs(adj, features, W1, b1, W2, b2, Wm1, bm1, Wm2, bm2)
    try:
        out, _ = _run(in_maps)
    except Exception:
        import time as _time
        _time.sleep(75)
        out, _ = _run(in_maps)
    return out


# revision 6
# speedup vs baseline: 2.8537x; 2.8537x over previous
"""GCN classifier on 8 TRN2 NeuronCores — 4-bit adjacency, SBUF-resident,
decoded to fp8 subnormal codes.

Math (reference):
    h1  = relu(adj @ (X @ W1) + b1)        [N, D]
    h2  = relu(adj @ (h1 @ W2) + b2)       [N, D]
    h3  = relu(h2 @ Wm1 + bm1)             [N, D]
    out = h3 @ Wm2 + bm2                   [N, 1]

Key ideas vs the uint8 predecessor:
  * adj is quantized to 4 bits (q = rint(a*15/amax), rel err ~2e-3 at the
    output) and packed 4 nibbles per u16.  The whole per-core shard is
    16.8 MB — it fits in SBUF, so it is DMA'd from HBM ONCE and both GCN
    layers unpack from the resident copy.  HBM adj traffic drops 4x vs
    uint8-streamed-twice.
  * fp8e4 (e4m3) bit patterns 0x00..0x0F decode linearly to n * 2^-9
    (subnormals + first binade), and the PE multiplies them exactly
    (hardware-verified).  So unpacking a nibble to an fp8 operand is a
    single AND — no magic-number offset, no colsum correction:
        ev pass:  (v & 0x0F0F)        -> two fp8 bytes per u16 lane
        od pass:  (v >> 4) & 0x0F0F   -> two more
    The 2^9 * amax/15 scale folds into the PSUM-evacuation activation.
  * Matmuls keep the predecessor's column-tiled structure (lhsT = Y
    k-blocks at PE tile positions (0,0)/(0,64); ev/od node halves on PSUM
    partition halves) which measures ~268 ns per k-block (all four
    512-wide matmuls overlap in the array).

Sharding: 1D row partition of adj (2048 rows/core), shard pre-transposed
on host to [N, P] nibbles.  All big matmuls contract over the SBUF
partition axis:

    layer 1:  Z1.T = (XW1).T @ B_c    (lhsT = host-computed XW1 k-blocks)
    gather:   per-core g = (h1 @ W2) blocks (PE-transposed), AllGather fp16
    layer 2:  Z2.T = G.T @ B_c
    head:     h3.T = relu(Wm1.T @ h2.T + bm1); out.T = Wm2.T @ h3.T + bm2

Layout bookkeeping: nibble-packing along P puts local nodes (4j..4j+3) in
lane j; the ev pass emits nodes (4j, 4j+2) at byte positions (2j, 2j+1)
and the od pass (4j+1, 4j+3) — i.e. outputs come out column-permuted
[evens | odds], identical to the predecessor.  The contraction (k) rows
of B_c and XW1 are host-permuted per 128-block to [evens | odds] so
layer-2's gathered G blocks line up.  The host unpermutes the final
[1, P] outputs.
"""

import numpy as np

N = 16384
D = 64
N_CORES = 8
P = N // N_CORES          # 2048 local nodes per core
KB = N // 128             # 128 contraction blocks
T_PER_CORE = P // 128     # 16 local row-blocks per core
LW = P // 4               # 512 u16 lanes per packed row
SLAB_KB = 8               # k-blocks per resident-load DMA slab
CH = 4                    # k-blocks unpacked per DVE chunk
EO_BUFS = 3               # unpacked ev/od chunk tiles in flight

_cache = {}


def _build(reps=1, use_collective=True, ablate=None):
    import concourse.bass as bass  # noqa: F401
    import concourse.mybir as mybir
    import concourse.tile as tile
    from concourse import bacc

    f32 = mybir.dt.float32
    fp16 = mybir.dt.float16
    fp8 = mybir.dt.float8e4
    u16 = mybir.dt.uint16

    AND = mybir.AluOpType.bitwise_and
    SHR = mybir.AluOpType.logical_shift_right
    relu = mybir.ActivationFunctionType.Relu
    ident = mybir.ActivationFunctionType.Identity
    copy = mybir.ActivationFunctionType.Copy

    nc = bacc.Bacc("TRN2", target_bir_lowering=False, debug=False,
                   num_devices=N_CORES)

    adjq = nc.dram_tensor("adjq", [N, LW], u16, kind="ExternalInput")
    yb = nc.dram_tensor("yb", [128, KB, D], fp16, kind="ExternalInput")
    w2 = nc.dram_tensor("w2", [D, D], fp16, kind="ExternalInput")
    wm1 = nc.dram_tensor("wm1", [D, D], fp16, kind="ExternalInput")
    wm2 = nc.dram_tensor("wm2", [D, 1], fp16, kind="ExternalInput")
    b1 = nc.dram_tensor("b1", [128, 1], f32, kind="ExternalInput")
    b2 = nc.dram_tensor("b2", [128, 1], f32, kind="ExternalInput")
    bm1 = nc.dram_tensor("bm1", [128, 1], f32, kind="ExternalInput")
    bm2 = nc.dram_tensor("bm2", [128, 1], f32, kind="ExternalInput")
    sc = nc.dram_tensor("sc", [128, 1], f32, kind="ExternalInput")  # 512*amax/15
    out = nc.dram_tensor("out", [2, P // 2], f32, kind="ExternalOutput")

    hb_ins = [nc.dram_tensor(f"hb_in{r}", [128, T_PER_CORE * D], fp16)
              for r in range(reps)]
    hb_outs = [nc.dram_tensor(f"hb_out{r}", [N_CORES * 128, T_PER_CORE * D],
                              fp16, addr_space="Shared")
               for r in range(reps)]

    n_slabs = KB // SLAB_KB
    n_chunks = KB // CH
    HP = P // 2                          # 1024
    HL = LW // 2                         # 256 u16 lanes per node-half

    with tile.TileContext(nc) as tc:
        with (
            tc.tile_pool(name="rq", bufs=1) as rq,       # resident packed adj
            tc.tile_pool(name="eo", bufs=EO_BUFS) as eo,
            tc.tile_pool(name="wq", bufs=1) as wq,       # yt / g_all (shared)
            tc.tile_pool(name="wpool", bufs=1) as wpool,
            tc.tile_pool(name="hv", bufs=1) as hv,
            tc.tile_pool(name="gpool", bufs=2) as gpool,
            tc.tile_pool(name="opool", bufs=1) as opool,
            tc.tile_pool(name="pacc", bufs=2, space="PSUM") as pacc,
            tc.tile_pool(name="psmall", bufs=1, space="PSUM") as psmall,
        ):
            # ---- constants (both partition halves) ----
            w2t = wpool.tile([128, D], fp16, tag="w2")
            nc.gpsimd.dma_start(w2t[0:D, :], w2[:])
            nc.gpsimd.dma_start(w2t[D:128, :], w2[:])
            wm1t = wpool.tile([128, D], fp16, tag="wm1")
            nc.gpsimd.dma_start(wm1t[0:D, :], wm1[:])
            nc.gpsimd.dma_start(wm1t[D:128, :], wm1[:])
            wm2t = wpool.tile([128, 1], fp16, tag="wm2")
            nc.gpsimd.dma_start(wm2t[0:D, :], wm2[:])
            nc.gpsimd.dma_start(wm2t[D:128, :], wm2[:])
            b1t = wpool.tile([128, 1], f32, tag="b1")
            nc.gpsimd.dma_start(b1t[:], b1[:])
            b2t = wpool.tile([128, 1], f32, tag="b2")
            nc.gpsimd.dma_start(b2t[:], b2[:])
            bm1t = wpool.tile([128, 1], f32, tag="bm1")
            nc.gpsimd.dma_start(bm1t[:], bm1[:])
            bm2t = wpool.tile([128, 1], f32, tag="bm2")
            nc.gpsimd.dma_start(bm2t[:], bm2[:])
            sct = wpool.tile([128, 1], f32, tag="sc")
            nc.gpsimd.dma_start(sct[:], sc[:])

            # ---- resident packed adjacency: loaded once, used twice ----
            rt = rq.tile([128, KB, LW], u16, tag="rq")
            if ablate == "dma":
                nc.vector.memset(rt[:, 0, :], 0x1234)
            else:
                for s in range(n_slabs):
                    src = adjq[s * SLAB_KB * 128:(s + 1) * SLAB_KB * 128, :]
                    nc.sync.dma_start(
                        rt[:, s * SLAB_KB:(s + 1) * SLAB_KB, :],
                        src.rearrange("(p n) i -> p n i", p=128))

            def big_layer(lhsT_of, psum_t, et0=None):
                """Unpack resident nibbles chunk-by-chunk; accumulate into
                psum_t [128, HP]: partitions 0:64 take the even-node output
                columns (ev), 64:128 the odd-node columns (od)."""
                for c in range(n_chunks):
                    if et0 is not None:
                        et = et0
                    else:
                        et = eo.tile([128, CH, 2 * LW], u16, tag="eo")
                        nc.vector.tensor_scalar(
                            et[:, :, 0:LW], rt[:, c * CH:(c + 1) * CH, :],
                            0x0F0F, None, AND)
                        if ablate != "dve1":
                            nc.vector.tensor_scalar(
                                et[:, :, LW:2 * LW],
                                rt[:, c * CH:(c + 1) * CH, :],
                                4, 0x0F0F, SHR, AND)
                    for j in range(CH):
                        kb = c * CH + j
                        first = kb == 0
                        last = kb == KB - 1
                        if ablate == "mm" and not (first or last):
                            continue
                        for ic in range(2):
                            nc.tensor.matmul(
                                psum_t[0:D, ic * 512:(ic + 1) * 512],
                                lhsT_of(kb),
                                et[:, j, ic * HL:(ic + 1) * HL].bitcast(fp8),
                                start=first, stop=last, tile_position=(0, 0))
                        for ic in range(2):
                            nc.tensor.matmul(
                                psum_t[D:128, ic * 512:(ic + 1) * 512],
                                lhsT_of(kb),
                                et[:, j, LW + ic * HL:LW + (ic + 1) * HL]
                                .bitcast(fp8),
                                start=first, stop=last, tile_position=(0, 64))

            def evac_relu(psum_t, bias_t, htag):
                """h block = relu(sc * psum + bias), kept in the two-half
                block layout [128, HP]."""
                ht = hv.tile([128, HP], fp16, tag=htag)
                nc.scalar.activation(ht[:], psum_t[:], relu, bias=bias_t[:],
                                     scale=sct[:])
                return ht

            for _rep in range(reps):
                hb_in = hb_ins[_rep]
                hb_out = hb_outs[_rep]

                yt = wq.tile([128, KB, D], fp16, tag="wq")
                nc.scalar.dma_start(yt[:], yb[:])

                et0 = None
                if ablate == "static":
                    et0 = gpool.tile([128, CH, 2 * LW], u16, tag="et0")
                    nc.vector.tensor_scalar(
                        et0[:, :, 0:LW], rt[:, 0:CH, :], 0x0F0F, None, AND)
                    nc.vector.tensor_scalar(
                        et0[:, :, LW:2 * LW], rt[:, 0:CH, :],
                        4, 0x0F0F, SHR, AND)

                # ---- layer 1 ----
                pz = pacc.tile([128, HP], f32, tag="acc")
                big_layer(lambda kb: yt[:, kb, :], pz, et0)
                h1 = evac_relu(pz, b1t, "h1")

                if ablate in ("l1", "l1g") :
                    outsb0 = opool.tile([128, HP], f32, tag="out")
                    nc.scalar.activation(outsb0[0:2, :], h1[0:2, :], ident,
                                         bias=bm2t[0:2, :])
                    nc.scalar.dma_start(out[:], outsb0[0:2, :])
                    if ablate == "l1":
                        continue

                # ---- local g blocks: g = h1 @ W2, PE-transposed ----
                gl = gpool.tile([128, T_PER_CORE, D], fp16, tag="gl")
                for t in range(T_PER_CORE):
                    pg = psmall.tile([128, D], f32, tag="pg")
                    nc.tensor.matmul(pg[0:D, :],
                                     h1[0:D, t * D:(t + 1) * D], w2t[0:D, :],
                                     start=True, stop=True,
                                     tile_position=(0, 0))
                    nc.tensor.matmul(pg[D:128, :],
                                     h1[D:128, t * D:(t + 1) * D],
                                     w2t[D:128, :], start=True, stop=True,
                                     tile_position=(64, 64))
                    nc.scalar.activation(gl[:, t, :], pg[:], copy)
                nc.scalar.dma_start(hb_in[:], gl[:])

                # ---- AllGather g ----
                if use_collective:
                    nc.gpsimd.collective_compute(
                        "AllGather",
                        mybir.AluOpType.bypass,
                        replica_groups=[list(range(N_CORES))],
                        ins=[hb_in.ap().opt()],
                        outs=[hb_out.ap().opt()],
                    )
                g_all = wq.tile([128, KB, D], fp16, tag="wq")
                for src in range(N_CORES):
                    if use_collective:
                        nc.scalar.dma_start(
                            g_all[:, src * T_PER_CORE:(src + 1) * T_PER_CORE,
                                  :],
                            hb_out[src * 128:(src + 1) * 128, :].rearrange(
                                "p (t d) -> p t d", d=D))
                    else:
                        nc.scalar.dma_start(
                            g_all[:, src * T_PER_CORE:(src + 1) * T_PER_CORE,
                                  :],
                            hb_in[:].rearrange("p (t d) -> p t d", d=D))

                if ablate == "l1g":
                    pz2x = psmall.tile([128, D], f32, tag="pg")
                    nc.tensor.matmul(pz2x[0:D, :], g_all[:, 0, :],
                                     w2t[0:D, :], start=True, stop=True)
                    nc.scalar.activation(gl[:, 0, :], pz2x[:], copy)
                    continue

                # ---- layer 2 ----
                pz2 = pacc.tile([128, HP], f32, tag="acc")
                big_layer(lambda kb: g_all[:, kb, :], pz2, et0)
                h2 = evac_relu(pz2, b2t, "h2")

                # ---- head (block layout: halves on partition halves) ----
                p3 = pacc.tile([128, HP], f32, tag="acc")
                for ic in range(2):
                    nc.tensor.matmul(p3[0:D, ic * 512:(ic + 1) * 512],
                                     wm1t[0:D, :],
                                     h2[0:D, ic * 512:(ic + 1) * 512],
                                     start=True, stop=True,
                                     tile_position=(0, 0))
                    nc.tensor.matmul(p3[D:128, ic * 512:(ic + 1) * 512],
                                     wm1t[D:128, :],
                                     h2[D:128, ic * 512:(ic + 1) * 512],
                                     start=True, stop=True,
                                     tile_position=(64, 64))
                h3 = hv.tile([128, HP], fp16, tag="h3")
                nc.scalar.activation(h3[:], p3[:], relu, bias=bm1t[:])

                outsb = opool.tile([128, HP], f32, tag="out")
                for ic in range(2):
                    po = psmall.tile([128, 512], f32, tag="po")
                    nc.tensor.matmul(po[0:1, :], wm2t[0:D, :],
                                     h3[0:D, ic * 512:(ic + 1) * 512],
                                     start=True, stop=True,
                                     tile_position=(0, 0))
                    nc.tensor.matmul(po[D:D + 1, :], wm2t[D:128, :],
                                     h3[D:128, ic * 512:(ic + 1) * 512],
                                     start=True, stop=True,
                                     tile_position=(64, 64))
                    nc.scalar.activation(outsb[0:1, ic * 512:(ic + 1) * 512],
                                         po[0:1, :], ident, bias=bm2t[0:1, :])
                    nc.scalar.activation(
                        outsb[D:D + 1, ic * 512:(ic + 1) * 512],
                        po[D:D + 1, :], ident, bias=bm2t[D:D + 1, :])
                nc.scalar.dma_start(out[:], outsb[0:128:D, :])

    nc.compile()
    return nc


def _perm_k():
    """Per-128-block [evens | odds] permutation of row indices."""
    i = np.arange(N)
    b, r = i // 128, i % 128
    return b * 128 + np.where(r < 64, 2 * r, 2 * (r - 64) + 1)


_SIGMA = None


def _sigma():
    """Output column m -> local node index."""
    global _SIGMA
    if _SIGMA is None:
        m = np.arange(P)
        _SIGMA = np.where(m < P // 2, 2 * m, 2 * (m - P // 2) + 1)
    return _SIGMA


def _shard_adjq(adj, scale_inv, pk, c):
    """Quantize to 4 bits + transpose + k-permute + slab-swizzle one
    core's shard.

    Column packing: u16 lane j holds local nodes 4j..4j+3 at nibble
    positions 0/4/8/12, so the device's ev pass (v & 0x0F0F) emits nodes
    (4j, 4j+2) and the od pass ((v>>4) & 0x0F0F) nodes (4j+1, 4j+3) —
    the [evens | odds] column order the rest of the pipeline assumes.

    Row swizzle: storage row s*G + p*SLAB_KB + j holds logical (permuted)
    row s*G + j*128 + p, so each partition's slab read is one contiguous
    SLAB_KB*LW*2-byte run (G = SLAB_KB*128 rows per slab)."""
    block = adj[c * P:(c + 1) * P, :]                  # [P, N] f32
    q16 = np.empty((N, LW), dtype=np.uint16)
    step = 2048
    for k0 in range(0, N, step):
        sub = block[:, k0:k0 + step].T * scale_inv     # [step, P]
        q = np.rint(sub).astype(np.uint16)             # 4-bit values
        q16[k0:k0 + step, :] = (q[:, 0::4] | (q[:, 1::4] << 4)
                                | (q[:, 2::4] << 8) | (q[:, 3::4] << 12))
    r = np.arange(N)
    G = SLAB_KB * 128
    s_, rem = r // G, r % G
    p_, j_ = rem // SLAB_KB, rem % SLAB_KB
    src = pk[s_ * G + j_ * 128 + p_]
    return np.ascontiguousarray(q16[src, :])           # [N, LW]


def _prep_inputs(adj, features, W1, b1, W2, b2, Wm1, bm1, Wm2, bm2):
    from concurrent.futures import ThreadPoolExecutor

    adj = np.asarray(adj, dtype=np.float32)
    amax = float(adj.max())
    scale_inv = 15.0 / amax
    pk = _perm_k()

    y = np.asarray(features, np.float32) @ np.asarray(W1, np.float32)
    y = y[pk, :]                                       # [N, 64] permuted
    ybv = np.ascontiguousarray(
        y.reshape(KB, 128, D).transpose(1, 0, 2)).astype(np.float16)

    def dup(v):
        v = np.asarray(v, np.float32).reshape(-1)
        if v.size == 1:
            return np.full((128, 1), v[0], np.float32)
        return np.concatenate([v, v]).reshape(128, 1)

    common = {
        "yb": ybv,
        "w2": np.asarray(W2, np.float16),
        "wm1": np.asarray(Wm1, np.float16),
        "wm2": np.asarray(Wm2, np.float16).reshape(D, 1),
        "b1": dup(b1),
        "b2": dup(b2),
        "bm1": dup(bm1),
        "bm2": dup(bm2),
        "sc": np.full((128, 1), 512.0 * amax / 15.0, np.float32),
    }
    with ThreadPoolExecutor(max_workers=8) as ex:
        shards = list(ex.map(
            lambda c: _shard_adjq(adj, scale_inv, pk, c), range(N_CORES)))
    return [dict(common, adjq=shards[c]) for c in range(N_CORES)]


def _run(in_maps, **kw):
    from concourse.bass_utils import run_bass_kernel_spmd

    if "nc" not in _cache:
        _cache["nc"] = _build()
    res = run_bass_kernel_spmd(_cache["nc"], in_maps,
                               core_ids=list(range(N_CORES)), **kw)
    sig = _sigma()
    full = np.empty((N,), np.float32)
    for c in range(N_CORES):
        full[c * P + sig] = res.results[c]["out"].reshape(-1)
    return full[:, None], res


def kernel(adj, features, W1, b1, W2, b2, Wm1, bm1, Wm2, bm2):
    in_maps = _prep_inputs(adj, features, W1, b1, W2, b2, Wm1, bm1, Wm2, bm2)
    try:
        out, _ = _run(in_maps)
    except Exception:
        import time as _time
        _time.sleep(75)
        out, _ = _run(in_maps)
    return out
